# revision 23
# baseline (speedup 1.0000x reference)
"""PointWarping v6: exact on-device 3-NN, index+score readback.

The axon tunnel costs one ~80-95ms protocol round trip per dispatched
call (independent of device count; independent calls do NOT pipeline),
with input upload and async output fetch largely overlapping that RTT.
So the design is: one device call, minimal host work on the critical
path after the wait.

Device per core (4096 queries x 8192 points, 32 tiles of 128 queries):
inputs are RAW coords qr [3,4096] (query half-batch) and kr [3,8192]
(warped database pos1+flow1 for the batch).  Scores s = q.k - |k|^2/2
(ranking by s descending == squared distance ascending, |q|^2 is a
per-row constant) come from one K=6 true-f32 matmul with lhsT rows
[q0,q1,q2,-.5,-.5,-.5] and rhs rows [k0,k1,k2,k0^2,k1^2,k2^2].  (f32r
matmul has ~1e-5 relative noise and misranks ~16% of rows; true f32
matches the reference ranking everywhere.)  Compute ops cannot start
at partition>0, so the -0.5 rows come from a full-tile memset
overwritten by the q DMA, and the squared rows are squared at
partition 0 then moved to partitions 3:6 by an SBUF->SBUF DMA.
Per tile, 8 PSUM chunks are ACT-drained to a [128,8192] f32 SBUF row;
one DVE max8 + one max_index give the exact top-8 values and columns
per query (ties resolve to distinct positions lowest-index-first,
matching jax.lax.top_k).  Both write [128,8] into overlapping windows
of persistent tiles so cols 3t..3t+3 keep tile t's top-3 (DVE executes
in program order, so tile t+1's overwrite of the 5 spare cols lands
after tile t's max_index read).  Output: [128,96] u16 indices +
[128,96] f32 scores per core, fetched async during the RTT.

Host after the wait: d2 = |q|^2 - 2s needs no coordinate gather; one
flow gather + inverse-distance weights + warp + clip, vectorized numpy
over all 8 cores (~4ms).  No re-rank, no rescue path.  Output
placeholder buffers are uploaded once and reused across calls (the
device program never reads them; in-graph jnp.zeros placeholders crash
the axon backend compile).
"""

import numpy as np

B, C, N = 4, 3, 8192
NQ = 4096
NT = 32
EPS = 1e-10
CLAMP = 10.0

_CACHE = {}


def _build():
    if "nc" in _CACHE:
        return _CACHE["nc"]

    from contextlib import ExitStack
    from concourse import bacc, bass, tile
    from concourse import mybir

    nc = bacc.Bacc("TRN2", target_bir_lowering=False, debug=False,
                   enable_asserts=True, num_devices=1)
    f32 = mybir.dt.float32
    u16 = mybir.dt.uint16
    MULT = mybir.AluOpType.mult

    qr = nc.dram_tensor("qr", [3, NQ], f32, kind="ExternalInput").ap()
    kr = nc.dram_tensor("kr", [3, N], f32, kind="ExternalInput").ap()
    # query-major [NQ, 3] layout so the host consumes them with zero-copy
    # reshapes (query 128t+p is row 128t+p via the per-tile DMAs below)
    gidxo = nc.dram_tensor("gidxo", [NQ, 3], u16,
                           kind="ExternalOutput").ap()
    gvalo = nc.dram_tensor("gvalo", [NQ, 3], f32,
                           kind="ExternalOutput").ap()

    with tile.TileContext(nc) as tc, ExitStack() as ctx:
        cp = ctx.enter_context(tc.tile_pool(name="persist", bufs=1))
        tp = ctx.enter_context(tc.tile_pool(name="loop", bufs=2))
        pp = ctx.enter_context(tc.tile_pool(name="ps", bufs=4, space="PSUM"))

        # s = q.k - |k|^2/2 via one K=6 contraction:
        #   lhsT rows = [q0,q1,q2,-.5,-.5,-.5], rhs rows = [k0,k1,k2,k0^2,
        #   k1^2,k2^2].  Compute ops cannot start at partition>0, so the
        #   constant rows come from a full-tile memset overwritten by the q
        #   DMA, and the squared rows are squared at partition 0 then moved
        #   to partitions 3:6 by an SBUF->SBUF DMA (DMA may target any
        #   partition).
        Q6 = cp.tile([6, NQ], f32, tag="Q6", bufs=1, name="Q6")
        K6 = cp.tile([6, N], f32, tag="K6", bufs=1, name="K6")
        KSQ = cp.tile([3, N], f32, tag="KSQ", bufs=1, name="KSQ")

        nc.vector.memset(Q6[:, :], -0.5)
        nc.sync.dma_start(Q6[0:3, 0:128], qr[:, 0:128])
        nc.sync.dma_start(K6[0:3, 0:2730], kr[:, 0:2730])
        nc.scalar.dma_start(K6[0:3, 2730:5461], kr[:, 2730:5461])
        nc.gpsimd.dma_start(K6[0:3, 5461:8192], kr[:, 5461:8192])
        nc.sync.dma_start(Q6[0:3, 128:NQ], qr[:, 128:NQ])

        nc.vector.tensor_tensor(KSQ[:, :], K6[0:3, :], K6[0:3, :], MULT)
        nc.sync.dma_start(K6[3:6, :], KSQ[:, :])

        # cols 3t..3t+3 hold tile t's top-3; max/max_index write 8 cols per
        # tile, the 5 extra are overwritten by the next tile (engine-ordered
        # on DVE), with 5 pad cols for the last tile.
        GIDX3 = cp.tile([128, 3 * NT + 5], u16, tag="GIDX3", bufs=1,
                        name="GIDX3")
        GVAL3 = cp.tile([128, 3 * NT + 5], f32, tag="GVAL3", bufs=1,
                        name="GVAL3")

        for t in range(NT):
            lhsT = Q6[:, bass.ts(t, 128)]
            S = tp.tile([128, N], f32, tag="S", name="S")
            for k in range(8):
                P = pp.tile([128, 1024], f32, tag="P", bufs=4, name=f"P{k}")
                for i in range(2):
                    nc.tensor.matmul(
                        P[:, bass.ts(i, 512)], lhsT,
                        K6[:, 1024 * k + 512 * i:1024 * k + 512 * (i + 1)],
                        start=True, stop=True)
                nc.scalar.copy(S[:, 1024 * k:1024 * (k + 1)], P[:, :])
            TOP = GVAL3[:, 3 * t:3 * t + 8]
            nc.vector.max(TOP, S[:, :])
            nc.vector.max_index(GIDX3[:, 3 * t:3 * t + 8], TOP, S[:, :])
            nc.sync.dma_start(gidxo[128 * t:128 * (t + 1), :],
                              GIDX3[:, 3 * t:3 * t + 3])
            nc.sync.dma_start(gvalo[128 * t:128 * (t + 1), :],
                              GVAL3[:, 3 * t:3 * t + 3])

    nc.compile()
    _CACHE["nc"] = nc
    return nc


def _get_runner():
    if "runner" in _CACHE:
        return _CACHE["runner"]

    import jax
    from jax.sharding import Mesh, PartitionSpec, NamedSharding
    import warnings
    with warnings.catch_warnings():
        warnings.simplefilter("ignore")
        try:
            from jax.experimental.shard_map import shard_map
        except ImportError:
            from jax import shard_map
    from concourse import mybir
    from concourse.bass2jax import (
        install_neuronx_cc_hook,
        _bass_exec_p,
        partition_id_tensor,
    )

    nc = _build()
    n_cores = 8
    install_neuronx_cc_hook()
    partition_name = (nc.partition_id_tensor.name
                      if nc.partition_id_tensor else None)

    in_names, out_names, out_avals = [], [], []
    for alloc in nc.m.functions[0].allocations:
        if not isinstance(alloc, mybir.MemoryLocationSet):
            continue
        name = alloc.memorylocations[0].name
        if alloc.kind == "ExternalInput":
            if name != partition_name:
                in_names.append(name)
        elif alloc.kind == "ExternalOutput":
            out_names.append(name)
            shape = tuple(alloc.tensor_shape)
            dtype = mybir.dt.np(alloc.dtype)
            out_avals.append(jax.core.ShapedArray(shape, dtype))
    all_names = list(in_names) + list(out_names)
    if partition_name is not None:
        all_names.append(partition_name)

    def _body(*args):
        operands = list(args)
        if partition_name is not None:
            operands.append(partition_id_tensor())
        outs = _bass_exec_p.bind(
            *operands,
            out_avals=tuple(out_avals),
            in_names=tuple(all_names),
            out_names=tuple(out_names),
            lowering_input_output_aliases=(),
            sim_require_finite=True,
            sim_require_nnan=True,
            nc=nc,
        )
        return tuple(outs)

    devices = jax.devices()[:n_cores]
    mesh = Mesh(np.asarray(devices), ("core",))
    in_specs = (PartitionSpec("core"),) * (len(in_names) + len(out_names))
    out_specs = (PartitionSpec("core"),) * len(out_names)
    try:
        smapped = shard_map(_body, mesh=mesh, in_specs=in_specs,
                            out_specs=out_specs, check_vma=False)
    except TypeError:
        smapped = shard_map(_body, mesh=mesh, in_specs=in_specs,
                            out_specs=out_specs, check_rep=False)
    sharded = jax.jit(smapped)

    # output placeholder buffers: uploaded once, reused every call (the
    # device program never reads them)
    sh = NamedSharding(mesh, PartitionSpec("core"))
    resident_zeros = [
        jax.device_put(
            np.zeros((n_cores * a.shape[0], *a.shape[1:]), a.dtype), sh)
        for a in out_avals
    ]
    for z in resident_zeros:
        z.block_until_ready()

    runner = {
        "sharded": sharded,
        "in_names": in_names,
        "out_names": out_names,
        "out_shapes": [(tuple(a.shape), a.dtype) for a in out_avals],
        "zeros": resident_zeros,
        "n_cores": n_cores,
    }
    _CACHE["runner"] = runner
    return runner


def _run_device(concat_in):
    """Run the bass kernel on 8 cores; returns a collect() closure."""

    r = _get_runner()
    n_cores = r["n_cores"]
    out = r["sharded"](*concat_in, *r["zeros"])
    for a in out:
        a.copy_to_host_async()

    def collect():
        return [np.asarray(a) for a in out]

    return collect


def kernel(pos1, pos2, flow1):
    pos1 = np.asarray(pos1, dtype=np.float32)
    pos2 = np.asarray(pos2, dtype=np.float32)
    flow1 = np.asarray(flow1, dtype=np.float32)

    # core c = 2b + h handles queries pos2[b, :, h*NQ:(h+1)*NQ] against the
    # full batch-b database k = pos1[b] + flow1[b]
    k_all = pos1 + flow1                                     # [4, 3, 8192]
    qr_in = np.concatenate(
        [pos2[c // 2, :, (c % 2) * NQ:(c % 2 + 1) * NQ] for c in range(8)],
        axis=0)                                              # [24, 4096]
    kr_in = np.concatenate([k_all[c // 2] for c in range(8)], axis=0)

    collect = _run_device([qr_in, kr_in])

    # host-side array prep overlaps the device round trip
    q_all = pos2.transpose(0, 2, 1).reshape(8, NQ, 3)        # [8, 4096, 3]
    qq = np.einsum("qnj,qnj->qn", q_all, q_all)              # |q|^2 [8, 4096]
    flow_flat = flow1.transpose(0, 2, 1).reshape(B * N, 3)
    base = (np.arange(8, dtype=np.int32) // 2 * N)[:, None, None]
    A = _CACHE.setdefault("scratchA", np.empty((8, NQ, 3), np.float32))

    gidx, gval = collect()                                   # [32768, 3] each
    # device wrote query-major rows: zero-copy reshapes
    idx = np.minimum(gidx.reshape(8, NQ, 3), N - 1) + base   # flat into [32768]

    # s = q.k - |k|^2/2  ->  d2 = |q|^2 - 2s; in-place chain down to 1/dist
    np.multiply(gval.reshape(8, NQ, 3), -2.0, out=A)
    np.add(A, qq[:, :, None], out=A)
    np.maximum(A, 0.0, out=A)
    np.sqrt(A, out=A)
    np.maximum(A, EPS, out=A)
    np.reciprocal(A, out=A)                                  # inv [8, 4096, 3]
    sinv = A[..., 0] + A[..., 1] + A[..., 2]
    # per-neighbor gathers avoid a [8,4096,3,3] temporary
    flow2 = A[..., 0, None] * flow_flat[idx[..., 0]]
    flow2 += A[..., 1, None] * flow_flat[idx[..., 1]]
    flow2 += A[..., 2, None] * flow_flat[idx[..., 2]]
    flow2 /= sinv[..., None]
    np.subtract(q_all, flow2, out=flow2)
    np.clip(flow2, -CLAMP, CLAMP, out=flow2)
    # [8, 4096, 3] -> [4, 3, 8192] with n = h*NQ + pos
    return np.ascontiguousarray(
        flow2.reshape(B, 2, NQ, 3).transpose(0, 3, 1, 2).reshape(B, C, N))
